# revision 6
# baseline (speedup 1.0000x reference)
"""GAT (2-layer, PyG-style) on 8 Trainium2 NeuronCores via Bass/Tile.

Strategy (dst-partition sharding per the hint):
- Host permutes nodes: globally degree-sorted, dealt round-robin to 8 cores so
  every core gets an identical block/degree profile (one shared NEFF per
  stage), padded to 98 blocks x 128 nodes per core. Edge slots are laid out
  [partition = dst node, free = slot] per block, padded with designated pad
  rows so segment reductions become dense axis reductions.
- K1 (device): per-node factor rows [h | a_src | a_dst] for this core's nodes
  via one matmul pass; the attention dot products are folded into a widened
  weight matrix (a_src = x @ (W per-head @ att_src)).
- Host staging: the per-edge gather. The runtime in this container executes
  dynamic/indirect DMA ~100x slower than spec (walrus: "DynamicDMA is
  disabled"), so the [h|a_src] row expansion to edge slots is staged on the
  host between NEFFs; the device consumes it as sequential streams.
- K2 (device): layer-1 edge phase per block: e = a_src + a_dst (broadcast),
  leaky-relu, exp WITHOUT segment-max (value range is safely small; softmax
  is shift-invariant), slot-axis sum for the denominator, alpha-weighted
  message sum (normalization deferred to node level, one reciprocal per
  node), bias + ELU, then the layer-2 factor row [h2 | a_src2 | a_dst2] via
  on-chip transpose + matmul.
- K3 (device): layer-2 edge phase -> scores.
- Host head: log-softmax / NLL / masked-mean loss / argmax.
"""
import sys

sys.path.insert(0, "/opt/trn_rl_repo")

import numpy as np

import concourse.bass as bass
import concourse.tile as tile
from concourse import mybir
from concourse.bass_utils import run_bass_kernel_spmd
from concourse.vector_clock import ScopedClock

F32 = mybir.dt.float32
I32 = mybir.dt.int32
Alu = mybir.AluOpType
Act = mybir.ActivationFunctionType
Ax = mybir.AxisListType

N = 100000
E = 1700000
IN = 128
H1, C1 = 8, 8
C2 = 64
NEG = 0.2
NCORES = 8
P = 128
NB = 98                      # blocks per core
NPC = NB * P                 # padded nodes per core (12544)
NTOT = NCORES * NPC          # padded total (100352)
R1 = 72                      # layer-1 slot row: h(64) | a_src(8)
R2 = 65                      # layer-2 slot row: h2(64) | a_src2(1)
PAD0 = NTOT - 2              # pad row with a_src = 0 (denominator anchor)
PADN = NTOT - 1              # pad row with a_src = -1e9 (zero contribution)
NEG_BIG = -1.0e9

MAX_WAITS_PER_INST = 1


# ---------------------------------------------------------------- tile patch
def _drain_and_barrier_split(self, tick_clock, wait_clock):
    """The walrus in this container rejects >1 sem-wait per instruction; split
    the Tile tail-drain's global-clock waits across sync-engine nops."""
    nc = self.nc
    probe = nc.sync.nop(nofuse=True, hint="drain_wait_probe")
    wait_clock.add_sem_waits(probe.ins, ScopedClock({None: tick_clock.global_clock}))
    si = probe.ins.sync_info
    waits = list(si.on_wait) if si is not None and si.on_wait else []
    if len(waits) > MAX_WAITS_PER_INST:
        si.on_wait[:] = waits[:MAX_WAITS_PER_INST]
        rest = waits[MAX_WAITS_PER_INST:]
        while rest:
            chunk, rest = rest[:MAX_WAITS_PER_INST], rest[MAX_WAITS_PER_INST:]
            extra = nc.sync.nop(nofuse=True, hint="drain_wait_split")
            esi = extra.ins.sync_info
            if esi is None:
                extra.ins.sync_info = mybir.SyncInfo(on_wait=list(chunk), on_update=[])
            else:
                esi.on_wait[:] = list(chunk)
    nc.sync.drain()
    nc.all_engine_barrier()
    assert self.sems is not None
    popped = nc._tile_sem_poison_stack.pop()
    assert popped is self._sem_poison
    nc.clear_and_free_semaphores(list(self.sems.allocated().values()))
    nc.all_engine_barrier()


tile.TileContext._drain_and_barrier = _drain_and_barrier_split


def _split_multiwait(nc, max_waits=MAX_WAITS_PER_INST):
    for func in nc.m.functions:
        for bb in func.blocks:
            new_list = []
            for inst in bb.instructions:
                si = inst.sync_info
                if si is not None and si.on_wait and len(si.on_wait) > max_waits:
                    waits = list(si.on_wait)
                    si.on_wait[:] = waits[-max_waits:]
                    rest = waits[:-max_waits]
                    k = 0
                    while rest:
                        chunk, rest = rest[:max_waits], rest[max_waits:]
                        nop = mybir.InstNoOp(name=f"{inst.name}-ws{k}", ins=[], outs=[])
                        nop.engine = inst.engine
                        nop.sync_info = mybir.SyncInfo(on_wait=list(chunk), on_update=[])
                        new_list.append(nop)
                        nc.register_instruction(nop, overwrite=True)
                        k += 1
                new_list.append(inst)
            bb.instructions[:] = new_list


def _ap(t_ap, extra_dims, offset=0):
    """Raw AP on a tile AP: keep its partition dim, replace the free dims."""
    return bass.AP(t_ap.tensor, t_ap.offset + offset, [t_ap.ap[0]] + extra_dims)


# ---------------------------------------------------------------- host prep
def _host_prep(feat, edge_index, W1, att_src1, att_dst1, W2, att_src2, att_dst2,
               b1, b2):
    srcg = np.asarray(edge_index[0])
    dstg = np.asarray(edge_index[1])
    deg = np.bincount(dstg, minlength=N)

    order = np.argsort(-deg, kind="stable")
    core_of = np.empty(N, np.int64)
    rank_of = np.empty(N, np.int64)
    idxs = np.arange(N)
    core_of[order] = idxs % NCORES
    rank_of[order] = idxs // NCORES
    perm = core_of * NPC + rank_of
    # block degree schedule (shared across all cores -> one NEFF)
    degP = np.zeros(NTOT, np.int64)
    degP[perm] = deg
    db = degP.reshape(NCORES, NB, P).max(axis=(0, 2))
    db = np.maximum(db, 1)
    boff = np.concatenate([[0], np.cumsum(db)]).astype(np.int64)
    S = int(boff[-1])

    # slot tables [core][128, S] of permuted src ids, default PADN
    srcP = perm[srcg]
    dstP = perm[dstg]
    eo = np.argsort(dstP, kind="stable")
    dstS = dstP[eo]
    srcS = srcP[eo]
    starts = np.zeros(NTOT + 1, np.int64)
    np.add.at(starts, dstS + 1, 1)
    starts = np.cumsum(starts)
    j = np.arange(E) - starts[dstS]
    core = dstS // NPC
    r = dstS % NPC
    col = boff[r // P] + j
    slots = np.full((NCORES, P, S), PADN, np.int32)
    slots[core, r % P, col] = srcS.astype(np.int32)
    padr = np.arange(N // NCORES, NPC)
    for c in range(NCORES):
        slots[c, padr % P, boff[padr // P]] = PAD0

    featP = np.zeros((NTOT, IN), np.float32)
    featP[perm] = np.asarray(feat, np.float32)

    W1 = np.asarray(W1, np.float32)
    Wa_src1 = np.stack(
        [W1[:, h * C1:(h + 1) * C1] @ np.asarray(att_src1, np.float32)[h]
         for h in range(H1)], axis=1)
    Wa_dst1 = np.stack(
        [W1[:, h * C1:(h + 1) * C1] @ np.asarray(att_dst1, np.float32)[h]
         for h in range(H1)], axis=1)
    W1cat = np.concatenate([W1, Wa_src1, Wa_dst1], axis=1)      # [128, 80]

    W2 = np.asarray(W2, np.float32)
    W2cat = np.concatenate(
        [W2,
         (W2 @ np.asarray(att_src2, np.float32)[0])[:, None],
         (W2 @ np.asarray(att_dst2, np.float32)[0])[:, None]], axis=1)  # [64, 66]

    b1b = np.broadcast_to(np.asarray(b1, np.float32)[None, :], (P, 64)).copy()
    b2b = np.broadcast_to(np.asarray(b2, np.float32)[None, :], (P, C2)).copy()

    return dict(perm=perm, db=db, boff=boff, S=S, slots=slots,
                featP=featP, W1cat=W1cat, W2cat=W2cat, b1b=b1b, b2b=b2b)


# ------------------------------------------------- K1: per-node factor rows
def _build_k1():
    nc = bass.Bass()
    featownT = nc.declare_dram_parameter("featownT", [IN, NPC], F32, isOutput=False)
    W1cat = nc.declare_dram_parameter("W1cat", [IN, 80], F32, isOutput=False)
    k1tab = nc.declare_dram_parameter("k1tab", [NPC, 80], F32, isOutput=True)

    with tile.TileContext(nc) as tc:
        with (
            tc.tile_pool(name="const", bufs=1) as constp,
            tc.tile_pool(name="lhs", bufs=3) as lhsp,
            tc.tile_pool(name="ps", bufs=2, space="PSUM") as psp,
            tc.tile_pool(name="sb", bufs=3) as sbp,
        ):
            w1_sb = constp.tile([IN, 80], F32)
            nc.sync.dma_start(out=w1_sb[:], in_=W1cat[:])
            G = 4
            ntile = NPC // P                       # 98
            for g in range((ntile + G - 1) // G):
                nblk = min(G, ntile - g * G)
                lhs = lhsp.tile([IN, P * G], F32, tag="lhs")
                nc.sync.dma_start(
                    out=lhs[:, :nblk * P],
                    in_=featownT[:, g * G * P: (g * G + nblk) * P])
                ps = psp.tile([P, G * 80], F32, tag="ps")
                for m in range(nblk):
                    nc.tensor.matmul(
                        out=ps[:, m * 80:(m + 1) * 80],
                        lhsT=lhs[:, m * P:(m + 1) * P],
                        rhs=w1_sb[:],
                        start=True, stop=True)
                sb = sbp.tile([P, G * 80], F32, tag="sb")
                nc.vector.tensor_copy(out=sb[:, :nblk * 80], in_=ps[:, :nblk * 80])
                dst = bass.AP(k1tab[:, :].tensor, g * G * P * 80,
                              [[80, P], [P * 80, nblk], [1, 80]])
                nc.sync.dma_start(out=dst, in_=sb[:, :nblk * 80])
    _split_multiwait(nc)
    return nc


# ---------------------------------------- K2: layer-1 edge phase + h2 rows
def _build_k2(db, S, rep=1):
    boff = np.concatenate([[0], np.cumsum(db)]).astype(np.int64)
    nc = bass.Bass()
    exp1 = nc.declare_dram_parameter("exp1", [P, S * R1], F32, isOutput=False)
    adstT = nc.declare_dram_parameter("adstT", [P, NB * 8], F32, isOutput=False)
    W2cat = nc.declare_dram_parameter("W2cat", [64, 66], F32, isOutput=False)
    b1b = nc.declare_dram_parameter("b1b", [P, 64], F32, isOutput=False)
    ident = nc.declare_dram_parameter("ident", [P, P], F32, isOutput=False)
    k1out = nc.declare_dram_parameter("k1out", [NPC, 66], F32, isOutput=True)

    with tile.TileContext(nc) as tc:
        with (
            tc.tile_pool(name="const", bufs=1) as constp,
            tc.tile_pool(name="gth", bufs=3) as gthp,
            tc.tile_pool(name="work", bufs=2) as workp,
            tc.tile_pool(name="ps", bufs=2, space="PSUM") as psp,
        ):
            w2_sb = constp.tile([64, 66], F32)
            nc.sync.dma_start(out=w2_sb[:], in_=W2cat[:])
            b1_sb = constp.tile([P, 64], F32)
            nc.sync.dma_start(out=b1_sb[:], in_=b1b[:])
            id_sb = constp.tile([P, P], F32)
            nc.sync.dma_start(out=id_sb[:], in_=ident[:])
            ad_sb = constp.tile([P, NB * 8], F32)
            nc.sync.dma_start(out=ad_sb[:], in_=adstT[:])

            for _rep in range(rep):
              for b in range(NB):
                d = int(db[b])
                off = int(boff[b])
                gth = gthp.tile([P, d * R1], F32, tag="gth")
                nc.sync.dma_start(
                    out=gth[:], in_=exp1[:, off * R1:(off + d) * R1])
                ga = gth[:, :]
                e = workp.tile([P, d * 8], F32, tag="e")
                nc.vector.tensor_tensor(
                    out=e[:],
                    in0=_ap(ga, [[R1, d], [1, 8]], offset=64),
                    in1=_ap(ad_sb[:, :], [[0, d], [1, 8]], offset=b * 8),
                    op=Alu.add)
                e2 = workp.tile([P, d * 8], F32, tag="e2")
                nc.vector.tensor_scalar_mul(e2[:], e[:], NEG)
                nc.vector.tensor_tensor(out=e2[:], in0=e[:], in1=e2[:], op=Alu.max)
                ex = workp.tile([P, d * 8], F32, tag="ex")
                nc.scalar.activation(out=ex[:], in_=e2[:], func=Act.Exp)
                den = workp.tile([P, 8], F32, tag="den")
                nc.vector.tensor_reduce(
                    out=den[:], in_=_ap(ex[:, :], [[1, 8], [8, d]]),
                    axis=Ax.X, op=Alu.add)
                rden = workp.tile([P, 8], F32, tag="rden")
                nc.vector.reciprocal(rden[:], den[:])
                msg = workp.tile([P, d * 64], F32, tag="msg")
                nc.vector.tensor_tensor(
                    out=msg[:],
                    in0=_ap(ga, [[R1, d], [1, 64]]),
                    in1=_ap(ex[:, :], [[8, d], [1, 8], [0, 8]]),
                    op=Alu.mult)
                oraw = workp.tile([P, 64], F32, tag="oraw")
                nc.vector.tensor_reduce(
                    out=oraw[:], in_=_ap(msg[:, :], [[1, 64], [64, d]]),
                    axis=Ax.X, op=Alu.add)
                x = workp.tile([P, 64], F32, tag="x")
                nc.vector.tensor_tensor(
                    out=x[:], in0=oraw[:],
                    in1=_ap(rden[:, :], [[1, 8], [0, 8]]), op=Alu.mult)
                nc.vector.tensor_tensor(out=x[:], in0=x[:], in1=b1_sb[:], op=Alu.add)
                # elu = exp(min(x,0)) - 1 + max(x,0)
                mn = workp.tile([P, 64], F32, tag="mn")
                nc.vector.tensor_scalar_min(mn[:], x[:], 0.0)
                em = workp.tile([P, 64], F32, tag="em")
                nc.scalar.activation(out=em[:], in_=mn[:], func=Act.Exp)
                nc.vector.tensor_scalar_max(x[:], x[:], 0.0)
                nc.vector.tensor_tensor(out=x[:], in0=x[:], in1=em[:], op=Alu.add)
                nc.vector.tensor_scalar_add(x[:], x[:], -1.0)
                # layer-2 factor row: [x @ W2 | x @ wa_src2 | x @ wa_dst2]
                xt_ps = psp.tile([64, P], F32, tag="xt")
                nc.tensor.transpose(out=xt_ps[:], in_=x[:], identity=id_sb[:])
                xt = workp.tile([64, P], F32, tag="xts")
                nc.vector.tensor_copy(out=xt[:], in_=xt_ps[:])
                h2 = psp.tile([P, 66], F32, tag="h2")
                nc.tensor.matmul(out=h2[:], lhsT=xt[:], rhs=w2_sb[:],
                                 start=True, stop=True)
                h2s = workp.tile([P, 66], F32, tag="h2s")
                nc.scalar.activation(out=h2s[:], in_=h2[:], func=Act.Copy)
                nc.sync.dma_start(out=k1out[b * P:(b + 1) * P, :], in_=h2s[:])
    _split_multiwait(nc)
    return nc


# ------------------------------------------------- K3: layer-2 edge phase
def _build_k3(db, S, rep=1):
    boff = np.concatenate([[0], np.cumsum(db)]).astype(np.int64)
    nc = bass.Bass()
    exp2 = nc.declare_dram_parameter("exp2", [P, S * R2], F32, isOutput=False)
    adst2T = nc.declare_dram_parameter("adst2T", [P, NB], F32, isOutput=False)
    b2b = nc.declare_dram_parameter("b2b", [P, C2], F32, isOutput=False)
    scout = nc.declare_dram_parameter("scout", [NPC, C2], F32, isOutput=True)

    with tile.TileContext(nc) as tc:
        with (
            tc.tile_pool(name="const", bufs=1) as constp,
            tc.tile_pool(name="gth", bufs=3) as gthp,
            tc.tile_pool(name="work", bufs=2) as workp,
        ):
            b2_sb = constp.tile([P, C2], F32)
            nc.sync.dma_start(out=b2_sb[:], in_=b2b[:])
            ad_sb = constp.tile([P, NB], F32)
            nc.sync.dma_start(out=ad_sb[:], in_=adst2T[:])

            for _rep in range(rep):
              for b in range(NB):
                d = int(db[b])
                off = int(boff[b])
                gth = gthp.tile([P, d * R2], F32, tag="gth")
                nc.sync.dma_start(
                    out=gth[:], in_=exp2[:, off * R2:(off + d) * R2])
                ga = gth[:, :]
                e = workp.tile([P, d], F32, tag="e")
                nc.vector.tensor_tensor(
                    out=e[:],
                    in0=_ap(ga, [[R2, d]], offset=64),
                    in1=_ap(ad_sb[:, :], [[0, d]], offset=b),
                    op=Alu.add)
                e2 = workp.tile([P, d], F32, tag="e2")
                nc.vector.tensor_scalar_mul(e2[:], e[:], NEG)
                nc.vector.tensor_tensor(out=e2[:], in0=e[:], in1=e2[:], op=Alu.max)
                ex = workp.tile([P, d], F32, tag="ex")
                nc.scalar.activation(out=ex[:], in_=e2[:], func=Act.Exp)
                den = workp.tile([P, 1], F32, tag="den")
                nc.vector.tensor_reduce(out=den[:], in_=ex[:], axis=Ax.X, op=Alu.add)
                rden = workp.tile([P, 1], F32, tag="rden")
                nc.vector.reciprocal(rden[:], den[:])
                msg = workp.tile([P, d * 64], F32, tag="msg")
                nc.vector.tensor_tensor(
                    out=msg[:],
                    in0=_ap(ga, [[R2, d], [1, 64]]),
                    in1=_ap(ex[:, :], [[1, d], [0, 64]]),
                    op=Alu.mult)
                oraw = workp.tile([P, 64], F32, tag="oraw")
                nc.vector.tensor_reduce(
                    out=oraw[:], in_=_ap(msg[:, :], [[1, 64], [64, d]]),
                    axis=Ax.X, op=Alu.add)
                sc = workp.tile([P, 64], F32, tag="sc")
                nc.vector.tensor_tensor(
                    out=sc[:], in0=oraw[:],
                    in1=_ap(rden[:, :], [[0, 64]]), op=Alu.mult)
                nc.vector.tensor_tensor(out=sc[:], in0=sc[:], in1=b2_sb[:], op=Alu.add)
                nc.sync.dma_start(out=scout[b * P:(b + 1) * P, :], in_=sc[:])
    _split_multiwait(nc)
    return nc


# ---------------------------------------------------------------- entry
def kernel(nodes, feat, edge_index, mask, label,
           W1, att_src1, att_dst1, b1, W2, att_src2, att_dst2, b2):
    prep = _host_prep(feat, edge_index, W1, att_src1, att_dst1,
                      W2, att_src2, att_dst2, b1, b2)
    db, S, slots = prep["db"], prep["S"], prep["slots"]
    featPT = np.ascontiguousarray(prep["featP"].T)
    ident = np.eye(P, dtype=np.float32)

    # K1: per-node [h | a_src | a_dst] rows (each core computes its own slice)
    nc1 = _build_k1()
    maps1 = [{
        "featownT": np.ascontiguousarray(featPT[:, c * NPC:(c + 1) * NPC]),
        "W1cat": prep["W1cat"],
    } for c in range(NCORES)]
    res1 = run_bass_kernel_spmd(nc1, maps1, list(range(NCORES)))
    k1tab = np.concatenate([res1.results[c]["k1tab"] for c in range(NCORES)], 0)

    # host staging: expand [h|a_src] rows to edge slots (dst-partitioned)
    table1 = np.ascontiguousarray(k1tab[:, :R1])
    table1[PAD0] = 0.0
    table1[PADN, :64] = 0.0
    table1[PADN, 64:] = NEG_BIG
    adstT = np.stack([
        np.ascontiguousarray(
            k1tab[c * NPC:(c + 1) * NPC, 72:80].reshape(NB, P, 8)
            .transpose(1, 0, 2).reshape(P, NB * 8))
        for c in range(NCORES)], 0)

    nc2 = _build_k2(db, S)
    maps2 = [{
        "exp1": table1[slots[c]].reshape(P, S * R1),
        "adstT": adstT[c],
        "W2cat": prep["W2cat"],
        "b1b": prep["b1b"],
        "ident": ident,
    } for c in range(NCORES)]
    res2 = run_bass_kernel_spmd(nc2, maps2, list(range(NCORES)))
    k1out = np.concatenate([res2.results[c]["k1out"] for c in range(NCORES)], 0)

    table2 = np.ascontiguousarray(k1out[:, :R2])
    table2[PAD0] = 0.0
    table2[PADN, :64] = 0.0
    table2[PADN, 64] = NEG_BIG
    adst2 = k1out[:, 65]

    nc3 = _build_k3(db, S)
    maps3 = [{
        "exp2": table2[slots[c]].reshape(P, S * R2),
        "adst2T": np.ascontiguousarray(
            adst2[c * NPC:(c + 1) * NPC].reshape(NB, P).T),
        "b2b": prep["b2b"],
    } for c in range(NCORES)]
    res3 = run_bass_kernel_spmd(nc3, maps3, list(range(NCORES)))
    scoresP = np.concatenate([res3.results[c]["scout"] for c in range(NCORES)], 0)
    scores = scoresP[prep["perm"]]

    # host head: log_softmax + masked-mean NLL + argmax
    m = scores.max(axis=1, keepdims=True)
    lse = (np.log(np.sum(np.exp(scores - m), axis=1, dtype=np.float32))
           + m[:, 0]).astype(np.float32)
    lab = np.asarray(label).astype(np.int32)
    nll = lse - scores[np.arange(N), lab]
    mf = np.asarray(mask).astype(np.float32)
    loss = np.float32(np.sum(nll * mf, dtype=np.float32)
                      / np.sum(mf, dtype=np.float32))
    pred = np.argmax(scores, axis=1).astype(np.int32)
    return loss, pred, lab


# revision 7
# speedup vs baseline: 1.0096x; 1.0096x over previous
"""GAT (2-layer, PyG-style) on 8 Trainium2 NeuronCores via Bass/Tile.

Strategy (dst-partition sharding per the hint):
- Host permutes nodes: globally degree-sorted, dealt round-robin to 8 cores so
  every core gets an identical block/degree profile (one shared NEFF per
  stage), padded to 98 blocks x 128 nodes per core. Edge slots are laid out
  [partition = dst node, free = slot] per block, padded with designated pad
  rows so segment reductions become dense axis reductions.
- K1 (device): per-node factor rows [h | a_src | a_dst] for this core's nodes
  via one matmul pass; the attention dot products are folded into a widened
  weight matrix (a_src = x @ (W per-head @ att_src)).
- Host staging: the per-edge gather. The runtime in this container executes
  dynamic/indirect DMA ~100x slower than spec (walrus: "DynamicDMA is
  disabled"), so the [h|a_src] row expansion to edge slots is staged on the
  host between NEFFs; the device consumes it as sequential streams.
- K2 (device): layer-1 edge phase per block: e = a_src + a_dst (broadcast),
  leaky-relu, exp WITHOUT segment-max (value range is safely small; softmax
  is shift-invariant), slot-axis sum for the denominator, alpha-weighted
  message sum (normalization deferred to node level, one reciprocal per
  node), bias + ELU, then the layer-2 factor row [h2 | a_src2 | a_dst2] via
  on-chip transpose + matmul.
- K3 (device): layer-2 edge phase -> scores.
- Host head: log-softmax / NLL / masked-mean loss / argmax.
"""
import sys

sys.path.insert(0, "/opt/trn_rl_repo")

import numpy as np

import concourse.bass as bass
import concourse.tile as tile
from concourse import mybir
from concourse.bass_utils import run_bass_kernel_spmd
from concourse.vector_clock import ScopedClock

F32 = mybir.dt.float32
I32 = mybir.dt.int32
Alu = mybir.AluOpType
Act = mybir.ActivationFunctionType
Ax = mybir.AxisListType

N = 100000
E = 1700000
IN = 128
H1, C1 = 8, 8
C2 = 64
NEG = 0.2
NCORES = 8
P = 128
NB = 98                      # blocks per core
NPC = NB * P                 # padded nodes per core (12544)
NTOT = NCORES * NPC          # padded total (100352)
R1 = 72                      # layer-1 slot row: h(64) | a_src(8)
R2 = 65                      # layer-2 slot row: h2(64) | a_src2(1)
PAD0 = NTOT - 2              # pad row with a_src = 0 (denominator anchor)
PADN = NTOT - 1              # pad row with a_src = -1e9 (zero contribution)
NEG_BIG = -1.0e9

MAX_WAITS_PER_INST = 1


# ---------------------------------------------------------------- tile patch
def _drain_and_barrier_split(self, tick_clock, wait_clock):
    """The walrus in this container rejects >1 sem-wait per instruction; split
    the Tile tail-drain's global-clock waits across sync-engine nops."""
    nc = self.nc
    probe = nc.sync.nop(nofuse=True, hint="drain_wait_probe")
    wait_clock.add_sem_waits(probe.ins, ScopedClock({None: tick_clock.global_clock}))
    si = probe.ins.sync_info
    waits = list(si.on_wait) if si is not None and si.on_wait else []
    if len(waits) > MAX_WAITS_PER_INST:
        si.on_wait[:] = waits[:MAX_WAITS_PER_INST]
        rest = waits[MAX_WAITS_PER_INST:]
        while rest:
            chunk, rest = rest[:MAX_WAITS_PER_INST], rest[MAX_WAITS_PER_INST:]
            extra = nc.sync.nop(nofuse=True, hint="drain_wait_split")
            esi = extra.ins.sync_info
            if esi is None:
                extra.ins.sync_info = mybir.SyncInfo(on_wait=list(chunk), on_update=[])
            else:
                esi.on_wait[:] = list(chunk)
    nc.sync.drain()
    nc.all_engine_barrier()
    assert self.sems is not None
    popped = nc._tile_sem_poison_stack.pop()
    assert popped is self._sem_poison
    nc.clear_and_free_semaphores(list(self.sems.allocated().values()))
    nc.all_engine_barrier()


tile.TileContext._drain_and_barrier = _drain_and_barrier_split


def _split_multiwait(nc, max_waits=MAX_WAITS_PER_INST):
    for func in nc.m.functions:
        for bb in func.blocks:
            new_list = []
            for inst in bb.instructions:
                si = inst.sync_info
                if si is not None and si.on_wait and len(si.on_wait) > max_waits:
                    waits = list(si.on_wait)
                    si.on_wait[:] = waits[-max_waits:]
                    rest = waits[:-max_waits]
                    k = 0
                    while rest:
                        chunk, rest = rest[:max_waits], rest[max_waits:]
                        nop = mybir.InstNoOp(name=f"{inst.name}-ws{k}", ins=[], outs=[])
                        nop.engine = inst.engine
                        nop.sync_info = mybir.SyncInfo(on_wait=list(chunk), on_update=[])
                        new_list.append(nop)
                        nc.register_instruction(nop, overwrite=True)
                        k += 1
                new_list.append(inst)
            bb.instructions[:] = new_list


def _ap(t_ap, extra_dims, offset=0):
    """Raw AP on a tile AP: keep its partition dim, replace the free dims."""
    return bass.AP(t_ap.tensor, t_ap.offset + offset, [t_ap.ap[0]] + extra_dims)


# ---------------------------------------------------------------- host prep
def _host_prep(feat, edge_index, W1, att_src1, att_dst1, W2, att_src2, att_dst2,
               b1, b2):
    srcg = np.asarray(edge_index[0])
    dstg = np.asarray(edge_index[1])
    deg = np.bincount(dstg, minlength=N)

    order = np.argsort(-deg, kind="stable")
    core_of = np.empty(N, np.int64)
    rank_of = np.empty(N, np.int64)
    idxs = np.arange(N)
    core_of[order] = idxs % NCORES
    rank_of[order] = idxs // NCORES
    perm = core_of * NPC + rank_of
    # block degree schedule (shared across all cores -> one NEFF)
    degP = np.zeros(NTOT, np.int64)
    degP[perm] = deg
    db = degP.reshape(NCORES, NB, P).max(axis=(0, 2))
    db = np.maximum(db, 1)
    boff = np.concatenate([[0], np.cumsum(db)]).astype(np.int64)
    S = int(boff[-1])

    # slot tables [core][128, S] of permuted src ids, default PADN
    srcP = perm[srcg]
    dstP = perm[dstg]
    eo = np.argsort(dstP, kind="stable")
    dstS = dstP[eo]
    srcS = srcP[eo]
    starts = np.zeros(NTOT + 1, np.int64)
    np.add.at(starts, dstS + 1, 1)
    starts = np.cumsum(starts)
    j = np.arange(E) - starts[dstS]
    core = dstS // NPC
    r = dstS % NPC
    col = boff[r // P] + j
    slots = np.full((NCORES, P, S), PADN, np.int32)
    slots[core, r % P, col] = srcS.astype(np.int32)
    padr = np.arange(N // NCORES, NPC)
    for c in range(NCORES):
        slots[c, padr % P, boff[padr // P]] = PAD0

    featP = np.zeros((NTOT, IN), np.float32)
    featP[perm] = np.asarray(feat, np.float32)

    W1 = np.asarray(W1, np.float32)
    Wa_src1 = np.stack(
        [W1[:, h * C1:(h + 1) * C1] @ np.asarray(att_src1, np.float32)[h]
         for h in range(H1)], axis=1)
    Wa_dst1 = np.stack(
        [W1[:, h * C1:(h + 1) * C1] @ np.asarray(att_dst1, np.float32)[h]
         for h in range(H1)], axis=1)
    W1cat = np.concatenate([W1, Wa_src1, Wa_dst1], axis=1)      # [128, 80]

    W2 = np.asarray(W2, np.float32)
    W2cat = np.concatenate(
        [W2,
         (W2 @ np.asarray(att_src2, np.float32)[0])[:, None],
         (W2 @ np.asarray(att_dst2, np.float32)[0])[:, None]], axis=1)  # [64, 66]

    b1b = np.broadcast_to(np.asarray(b1, np.float32)[None, :], (P, 64)).copy()
    b2b = np.broadcast_to(np.asarray(b2, np.float32)[None, :], (P, C2)).copy()

    return dict(perm=perm, db=db, boff=boff, S=S, slots=slots,
                featP=featP, W1cat=W1cat, W2cat=W2cat, b1b=b1b, b2b=b2b)


# ------------------------------------------------- K1: per-node factor rows
def _build_k1():
    nc = bass.Bass()
    featownT = nc.declare_dram_parameter("featownT", [IN, NPC], F32, isOutput=False)
    W1cat = nc.declare_dram_parameter("W1cat", [IN, 80], F32, isOutput=False)
    k1tab = nc.declare_dram_parameter("k1tab", [NPC, 80], F32, isOutput=True)

    with tile.TileContext(nc) as tc:
        with (
            tc.tile_pool(name="const", bufs=1) as constp,
            tc.tile_pool(name="lhs", bufs=3) as lhsp,
            tc.tile_pool(name="ps", bufs=2, space="PSUM") as psp,
            tc.tile_pool(name="sb", bufs=3) as sbp,
        ):
            w1_sb = constp.tile([IN, 80], F32)
            nc.sync.dma_start(out=w1_sb[:], in_=W1cat[:])
            G = 4
            ntile = NPC // P                       # 98
            for g in range((ntile + G - 1) // G):
                nblk = min(G, ntile - g * G)
                lhs = lhsp.tile([IN, P * G], F32, tag="lhs")
                nc.sync.dma_start(
                    out=lhs[:, :nblk * P],
                    in_=featownT[:, g * G * P: (g * G + nblk) * P])
                ps = psp.tile([P, G * 80], F32, tag="ps")
                for m in range(nblk):
                    nc.tensor.matmul(
                        out=ps[:, m * 80:(m + 1) * 80],
                        lhsT=lhs[:, m * P:(m + 1) * P],
                        rhs=w1_sb[:],
                        start=True, stop=True)
                sb = sbp.tile([P, G * 80], F32, tag="sb")
                nc.vector.tensor_copy(out=sb[:, :nblk * 80], in_=ps[:, :nblk * 80])
                dst = bass.AP(k1tab[:, :].tensor, g * G * P * 80,
                              [[80, P], [P * 80, nblk], [1, 80]])
                nc.sync.dma_start(out=dst, in_=sb[:, :nblk * 80])
    _split_multiwait(nc)
    return nc


# ---------------------------------------- K2: layer-1 edge phase + h2 rows
def _build_k2(db, S, rep=1):
    boff = np.concatenate([[0], np.cumsum(db)]).astype(np.int64)
    nc = bass.Bass()
    exp1 = nc.declare_dram_parameter("exp1", [P, S * R1], F32, isOutput=False)
    adstT = nc.declare_dram_parameter("adstT", [P, NB * 8], F32, isOutput=False)
    W2cat = nc.declare_dram_parameter("W2cat", [64, 66], F32, isOutput=False)
    b1b = nc.declare_dram_parameter("b1b", [P, 64], F32, isOutput=False)
    ident = nc.declare_dram_parameter("ident", [P, P], F32, isOutput=False)
    k1out = nc.declare_dram_parameter("k1out", [NPC, 66], F32, isOutput=True)

    with tile.TileContext(nc) as tc:
        with (
            tc.tile_pool(name="const", bufs=1) as constp,
            tc.tile_pool(name="gth", bufs=3) as gthp,
            tc.tile_pool(name="work", bufs=2) as workp,
            tc.tile_pool(name="ps", bufs=2, space="PSUM") as psp,
        ):
            w2_sb = constp.tile([64, 66], F32)
            nc.sync.dma_start(out=w2_sb[:], in_=W2cat[:])
            b1_sb = constp.tile([P, 64], F32)
            nc.sync.dma_start(out=b1_sb[:], in_=b1b[:])
            id_sb = constp.tile([P, P], F32)
            nc.sync.dma_start(out=id_sb[:], in_=ident[:])
            ad_sb = constp.tile([P, NB * 8], F32)
            nc.sync.dma_start(out=ad_sb[:], in_=adstT[:])

            for _rep in range(rep):
              for b in range(NB):
                d = int(db[b])
                off = int(boff[b])
                gth = gthp.tile([P, d * R1], F32, tag="gth")
                nc.sync.dma_start(
                    out=gth[:], in_=exp1[:, off * R1:(off + d) * R1])
                ga = gth[:, :]
                e = workp.tile([P, d * 8], F32, tag="e")
                nc.vector.tensor_tensor(
                    out=e[:],
                    in0=_ap(ga, [[R1, d], [1, 8]], offset=64),
                    in1=_ap(ad_sb[:, :], [[0, d], [1, 8]], offset=b * 8),
                    op=Alu.add)
                e2 = workp.tile([P, d * 8], F32, tag="e2")
                nc.vector.tensor_scalar_mul(e2[:], e[:], NEG)
                nc.vector.tensor_tensor(out=e2[:], in0=e[:], in1=e2[:], op=Alu.max)
                ex = workp.tile([P, d * 8], F32, tag="ex")
                nc.scalar.activation(out=ex[:], in_=e2[:], func=Act.Exp)
                den = workp.tile([P, 8], F32, tag="den")
                nc.vector.tensor_reduce(
                    out=den[:], in_=_ap(ex[:, :], [[1, 8], [8, d]]),
                    axis=Ax.X, op=Alu.add)
                rden = workp.tile([P, 8], F32, tag="rden")
                nc.vector.reciprocal(rden[:], den[:])
                msg = workp.tile([P, d * 64], F32, tag="msg")
                nc.vector.tensor_tensor(
                    out=msg[:],
                    in0=_ap(ga, [[R1, d], [1, 64]]),
                    in1=_ap(ex[:, :], [[8, d], [1, 8], [0, 8]]),
                    op=Alu.mult)
                oraw = workp.tile([P, 64], F32, tag="oraw")
                nc.vector.tensor_reduce(
                    out=oraw[:], in_=_ap(msg[:, :], [[1, 64], [64, d]]),
                    axis=Ax.X, op=Alu.add)
                x = workp.tile([P, 64], F32, tag="x")
                nc.vector.tensor_tensor(
                    out=x[:], in0=oraw[:],
                    in1=_ap(rden[:, :], [[1, 8], [0, 8]]), op=Alu.mult)
                nc.vector.tensor_tensor(out=x[:], in0=x[:], in1=b1_sb[:], op=Alu.add)
                # elu = exp(min(x,0)) - 1 + max(x,0)
                mn = workp.tile([P, 64], F32, tag="mn")
                nc.vector.tensor_scalar_min(mn[:], x[:], 0.0)
                em = workp.tile([P, 64], F32, tag="em")
                nc.scalar.activation(out=em[:], in_=mn[:], func=Act.Exp)
                nc.vector.tensor_scalar_max(x[:], x[:], 0.0)
                nc.vector.tensor_tensor(out=x[:], in0=x[:], in1=em[:], op=Alu.add)
                nc.vector.tensor_scalar_add(x[:], x[:], -1.0)
                # layer-2 factor row: [x @ W2 | x @ wa_src2 | x @ wa_dst2]
                xt_ps = psp.tile([64, P], F32, tag="xt")
                nc.tensor.transpose(out=xt_ps[:], in_=x[:], identity=id_sb[:])
                xt = workp.tile([64, P], F32, tag="xts")
                nc.vector.tensor_copy(out=xt[:], in_=xt_ps[:])
                h2 = psp.tile([P, 66], F32, tag="h2")
                nc.tensor.matmul(out=h2[:], lhsT=xt[:], rhs=w2_sb[:],
                                 start=True, stop=True)
                h2s = workp.tile([P, 66], F32, tag="h2s")
                nc.scalar.activation(out=h2s[:], in_=h2[:], func=Act.Copy)
                nc.sync.dma_start(out=k1out[b * P:(b + 1) * P, :], in_=h2s[:])
    _split_multiwait(nc)
    return nc


# ------------------------------------------------- K3: layer-2 edge phase
def _build_k3(db, S, rep=1):
    boff = np.concatenate([[0], np.cumsum(db)]).astype(np.int64)
    nc = bass.Bass()
    exp2 = nc.declare_dram_parameter("exp2", [P, S * R2], F32, isOutput=False)
    adst2T = nc.declare_dram_parameter("adst2T", [P, NB], F32, isOutput=False)
    b2b = nc.declare_dram_parameter("b2b", [P, C2], F32, isOutput=False)
    scout = nc.declare_dram_parameter("scout", [NPC, C2], F32, isOutput=True)

    with tile.TileContext(nc) as tc:
        with (
            tc.tile_pool(name="const", bufs=1) as constp,
            tc.tile_pool(name="gth", bufs=4) as gthp,
            tc.tile_pool(name="work", bufs=2) as workp,
        ):
            b2_sb = constp.tile([P, C2], F32)
            nc.sync.dma_start(out=b2_sb[:], in_=b2b[:])
            ad_sb = constp.tile([P, NB], F32)
            nc.sync.dma_start(out=ad_sb[:], in_=adst2T[:])

            for _rep in range(rep):
              for b in range(NB):
                d = int(db[b])
                off = int(boff[b])
                gth = gthp.tile([P, d * R2], F32, tag="gth")
                nc.sync.dma_start(
                    out=gth[:], in_=exp2[:, off * R2:(off + d) * R2])
                ga = gth[:, :]
                e = workp.tile([P, d], F32, tag="e")
                nc.vector.tensor_tensor(
                    out=e[:],
                    in0=_ap(ga, [[R2, d]], offset=64),
                    in1=_ap(ad_sb[:, :], [[0, d]], offset=b),
                    op=Alu.add)
                e2 = workp.tile([P, d], F32, tag="e2")
                nc.vector.tensor_scalar_mul(e2[:], e[:], NEG)
                nc.vector.tensor_tensor(out=e2[:], in0=e[:], in1=e2[:], op=Alu.max)
                ex = workp.tile([P, d], F32, tag="ex")
                nc.scalar.activation(out=ex[:], in_=e2[:], func=Act.Exp)
                den = workp.tile([P, 1], F32, tag="den")
                nc.vector.tensor_reduce(out=den[:], in_=ex[:], axis=Ax.X, op=Alu.add)
                rden = workp.tile([P, 1], F32, tag="rden")
                nc.vector.reciprocal(rden[:], den[:])
                msg = workp.tile([P, d * 64], F32, tag="msg")
                nc.vector.tensor_tensor(
                    out=msg[:],
                    in0=_ap(ga, [[R2, d], [1, 64]]),
                    in1=_ap(ex[:, :], [[1, d], [0, 64]]),
                    op=Alu.mult)
                oraw = workp.tile([P, 64], F32, tag="oraw")
                nc.vector.tensor_reduce(
                    out=oraw[:], in_=_ap(msg[:, :], [[1, 64], [64, d]]),
                    axis=Ax.X, op=Alu.add)
                sc = workp.tile([P, 64], F32, tag="sc")
                nc.vector.tensor_tensor(
                    out=sc[:], in0=oraw[:],
                    in1=_ap(rden[:, :], [[0, 64]]), op=Alu.mult)
                nc.vector.tensor_tensor(out=sc[:], in0=sc[:], in1=b2_sb[:], op=Alu.add)
                nc.sync.dma_start(out=scout[b * P:(b + 1) * P, :], in_=sc[:])
    _split_multiwait(nc)
    return nc


# ---------------------------------------------------------------- entry
def kernel(nodes, feat, edge_index, mask, label,
           W1, att_src1, att_dst1, b1, W2, att_src2, att_dst2, b2):
    prep = _host_prep(feat, edge_index, W1, att_src1, att_dst1,
                      W2, att_src2, att_dst2, b1, b2)
    db, S, slots = prep["db"], prep["S"], prep["slots"]
    featPT = np.ascontiguousarray(prep["featP"].T)
    ident = np.eye(P, dtype=np.float32)

    # K1: per-node [h | a_src | a_dst] rows (each core computes its own slice)
    nc1 = _build_k1()
    maps1 = [{
        "featownT": np.ascontiguousarray(featPT[:, c * NPC:(c + 1) * NPC]),
        "W1cat": prep["W1cat"],
    } for c in range(NCORES)]
    res1 = run_bass_kernel_spmd(nc1, maps1, list(range(NCORES)))
    k1tab = np.concatenate([res1.results[c]["k1tab"] for c in range(NCORES)], 0)

    # host staging: expand [h|a_src] rows to edge slots (dst-partitioned)
    table1 = np.ascontiguousarray(k1tab[:, :R1])
    table1[PAD0] = 0.0
    table1[PADN, :64] = 0.0
    table1[PADN, 64:] = NEG_BIG
    adstT = np.stack([
        np.ascontiguousarray(
            k1tab[c * NPC:(c + 1) * NPC, 72:80].reshape(NB, P, 8)
            .transpose(1, 0, 2).reshape(P, NB * 8))
        for c in range(NCORES)], 0)

    nc2 = _build_k2(db, S)
    maps2 = [{
        "exp1": table1[slots[c]].reshape(P, S * R1),
        "adstT": adstT[c],
        "W2cat": prep["W2cat"],
        "b1b": prep["b1b"],
        "ident": ident,
    } for c in range(NCORES)]
    res2 = run_bass_kernel_spmd(nc2, maps2, list(range(NCORES)))
    k1out = np.concatenate([res2.results[c]["k1out"] for c in range(NCORES)], 0)

    table2 = np.ascontiguousarray(k1out[:, :R2])
    table2[PAD0] = 0.0
    table2[PADN, :64] = 0.0
    table2[PADN, 64] = NEG_BIG
    adst2 = k1out[:, 65]

    nc3 = _build_k3(db, S)
    maps3 = [{
        "exp2": table2[slots[c]].reshape(P, S * R2),
        "adst2T": np.ascontiguousarray(
            adst2[c * NPC:(c + 1) * NPC].reshape(NB, P).T),
        "b2b": prep["b2b"],
    } for c in range(NCORES)]
    res3 = run_bass_kernel_spmd(nc3, maps3, list(range(NCORES)))
    scoresP = np.concatenate([res3.results[c]["scout"] for c in range(NCORES)], 0)
    scores = scoresP[prep["perm"]]

    # host head: log_softmax + masked-mean NLL + argmax
    m = scores.max(axis=1, keepdims=True)
    lse = (np.log(np.sum(np.exp(scores - m), axis=1, dtype=np.float32))
           + m[:, 0]).astype(np.float32)
    lab = np.asarray(label).astype(np.int32)
    nll = lse - scores[np.arange(N), lab]
    mf = np.asarray(mask).astype(np.float32)
    loss = np.float32(np.sum(nll * mf, dtype=np.float32)
                      / np.sum(mf, dtype=np.float32))
    pred = np.argmax(scores, axis=1).astype(np.int32)
    return loss, pred, lab


# revision 8
# speedup vs baseline: 1.0619x; 1.0518x over previous
"""GAT (2-layer, PyG-style) on 8 Trainium2 NeuronCores via Bass/Tile.

Strategy (dst-partition sharding per the hint):
- Host permutes nodes: globally degree-sorted, dealt round-robin to 8 cores so
  every core gets an identical block/degree profile (one shared NEFF per
  stage), padded to 98 blocks x 128 nodes per core. Edge slots are laid out
  [partition = dst node, free = slot] per block, padded with designated pad
  rows so segment reductions become dense axis reductions.
- K1 (device): per-node factor rows [h | a_src | a_dst] for this core's nodes
  via one matmul pass; the attention dot products are folded into a widened
  weight matrix (a_src = x @ (W per-head @ att_src)).
- Host staging: the per-edge gather. The runtime in this container executes
  dynamic/indirect DMA ~100x slower than spec (walrus: "DynamicDMA is
  disabled"), so the [h|a_src] row expansion to edge slots is staged on the
  host between NEFFs; the device consumes it as sequential streams.
- K2 (device): layer-1 edge phase per block: e = a_src + a_dst (broadcast),
  leaky-relu, exp WITHOUT segment-max (value range is safely small; softmax
  is shift-invariant), slot-axis sum for the denominator, alpha-weighted
  message sum (normalization deferred to node level, one reciprocal per
  node), bias + ELU, then the layer-2 factor row [h2 | a_src2 | a_dst2] via
  on-chip transpose + matmul.
- K3 (device): layer-2 edge phase -> scores.
- Host head: log-softmax / NLL / masked-mean loss / argmax.
"""
import sys

sys.path.insert(0, "/opt/trn_rl_repo")

import numpy as np

import concourse.bass as bass
import concourse.tile as tile
from concourse import mybir
from concourse.bass_utils import run_bass_kernel_spmd
from concourse.vector_clock import ScopedClock

F32 = mybir.dt.float32
I32 = mybir.dt.int32
Alu = mybir.AluOpType
Act = mybir.ActivationFunctionType
Ax = mybir.AxisListType

N = 100000
E = 1700000
IN = 128
H1, C1 = 8, 8
C2 = 64
NEG = 0.2
NCORES = 8
P = 128
NB = 98                      # blocks per core
NPC = NB * P                 # padded nodes per core (12544)
NTOT = NCORES * NPC          # padded total (100352)
R1 = 72                      # layer-1 slot row: h(64) | a_src(8)
R2 = 65                      # layer-2 slot row: h2(64) | a_src2(1)
PAD0 = NTOT - 2              # pad row with a_src = 0 (denominator anchor)
PADN = NTOT - 1              # pad row with a_src = -1e9 (zero contribution)
NEG_BIG = -1.0e9

MAX_WAITS_PER_INST = 1


# ---------------------------------------------------------------- tile patch
def _drain_and_barrier_split(self, tick_clock, wait_clock):
    """The walrus in this container rejects >1 sem-wait per instruction; split
    the Tile tail-drain's global-clock waits across sync-engine nops."""
    nc = self.nc
    probe = nc.sync.nop(nofuse=True, hint="drain_wait_probe")
    wait_clock.add_sem_waits(probe.ins, ScopedClock({None: tick_clock.global_clock}))
    si = probe.ins.sync_info
    waits = list(si.on_wait) if si is not None and si.on_wait else []
    if len(waits) > MAX_WAITS_PER_INST:
        si.on_wait[:] = waits[:MAX_WAITS_PER_INST]
        rest = waits[MAX_WAITS_PER_INST:]
        while rest:
            chunk, rest = rest[:MAX_WAITS_PER_INST], rest[MAX_WAITS_PER_INST:]
            extra = nc.sync.nop(nofuse=True, hint="drain_wait_split")
            esi = extra.ins.sync_info
            if esi is None:
                extra.ins.sync_info = mybir.SyncInfo(on_wait=list(chunk), on_update=[])
            else:
                esi.on_wait[:] = list(chunk)
    nc.sync.drain()
    nc.all_engine_barrier()
    assert self.sems is not None
    popped = nc._tile_sem_poison_stack.pop()
    assert popped is self._sem_poison
    nc.clear_and_free_semaphores(list(self.sems.allocated().values()))
    nc.all_engine_barrier()


tile.TileContext._drain_and_barrier = _drain_and_barrier_split


def _split_multiwait(nc, max_waits=MAX_WAITS_PER_INST):
    for func in nc.m.functions:
        for bb in func.blocks:
            new_list = []
            for inst in bb.instructions:
                si = inst.sync_info
                if si is not None and si.on_wait and len(si.on_wait) > max_waits:
                    waits = list(si.on_wait)
                    si.on_wait[:] = waits[-max_waits:]
                    rest = waits[:-max_waits]
                    k = 0
                    while rest:
                        chunk, rest = rest[:max_waits], rest[max_waits:]
                        nop = mybir.InstNoOp(name=f"{inst.name}-ws{k}", ins=[], outs=[])
                        nop.engine = inst.engine
                        nop.sync_info = mybir.SyncInfo(on_wait=list(chunk), on_update=[])
                        new_list.append(nop)
                        nc.register_instruction(nop, overwrite=True)
                        k += 1
                new_list.append(inst)
            bb.instructions[:] = new_list


def _ap(t_ap, extra_dims, offset=0):
    """Raw AP on a tile AP: keep its partition dim, replace the free dims."""
    return bass.AP(t_ap.tensor, t_ap.offset + offset, [t_ap.ap[0]] + extra_dims)


# ---------------------------------------------------------------- host prep
def _host_prep(feat, edge_index, W1, att_src1, att_dst1, W2, att_src2, att_dst2,
               b1, b2):
    srcg = np.asarray(edge_index[0])
    dstg = np.asarray(edge_index[1])
    deg = np.bincount(dstg, minlength=N)

    order = np.argsort(-deg, kind="stable")
    core_of = np.empty(N, np.int64)
    rank_of = np.empty(N, np.int64)
    idxs = np.arange(N)
    core_of[order] = idxs % NCORES
    rank_of[order] = idxs // NCORES
    perm = core_of * NPC + rank_of
    # block degree schedule (shared across all cores -> one NEFF)
    degP = np.zeros(NTOT, np.int64)
    degP[perm] = deg
    db = degP.reshape(NCORES, NB, P).max(axis=(0, 2))
    db = np.maximum(db, 1)
    boff = np.concatenate([[0], np.cumsum(db)]).astype(np.int64)
    S = int(boff[-1])

    # slot tables [core][128, S] of permuted src ids, default PADN
    srcP = perm[srcg]
    dstP = perm[dstg]
    eo = np.argsort(dstP, kind="stable")
    dstS = dstP[eo]
    srcS = srcP[eo]
    starts = np.zeros(NTOT + 1, np.int64)
    np.add.at(starts, dstS + 1, 1)
    starts = np.cumsum(starts)
    j = np.arange(E) - starts[dstS]
    core = dstS // NPC
    r = dstS % NPC
    col = boff[r // P] + j
    slots = np.full((NCORES, P, S), PADN, np.int32)
    slots[core, r % P, col] = srcS.astype(np.int32)
    padr = np.arange(N // NCORES, NPC)
    for c in range(NCORES):
        slots[c, padr % P, boff[padr // P]] = PAD0

    featP = np.zeros((NTOT, IN), np.float32)
    featP[perm] = np.asarray(feat, np.float32)

    W1 = np.asarray(W1, np.float32)
    Wa_src1 = np.stack(
        [W1[:, h * C1:(h + 1) * C1] @ np.asarray(att_src1, np.float32)[h]
         for h in range(H1)], axis=1)
    Wa_dst1 = np.stack(
        [W1[:, h * C1:(h + 1) * C1] @ np.asarray(att_dst1, np.float32)[h]
         for h in range(H1)], axis=1)
    W1cat = np.concatenate([W1, Wa_src1, Wa_dst1], axis=1)      # [128, 80]

    W2 = np.asarray(W2, np.float32)
    W2cat = np.concatenate(
        [W2,
         (W2 @ np.asarray(att_src2, np.float32)[0])[:, None],
         (W2 @ np.asarray(att_dst2, np.float32)[0])[:, None]], axis=1)  # [64, 66]

    b1b = np.broadcast_to(np.asarray(b1, np.float32)[None, :], (P, 64)).copy()
    b2b = np.broadcast_to(np.asarray(b2, np.float32)[None, :], (P, C2)).copy()

    return dict(perm=perm, db=db, boff=boff, S=S, slots=slots,
                featP=featP, W1cat=W1cat, W2cat=W2cat, b1b=b1b, b2b=b2b)


# ------------------------------------------------- K1: per-node factor rows
def _build_k1():
    nc = bass.Bass()
    featownT = nc.declare_dram_parameter("featownT", [IN, NPC], F32, isOutput=False)
    W1cat = nc.declare_dram_parameter("W1cat", [IN, 80], F32, isOutput=False)
    k1tab = nc.declare_dram_parameter("k1tab", [NPC, 80], F32, isOutput=True)

    with tile.TileContext(nc) as tc:
        with (
            tc.tile_pool(name="const", bufs=1) as constp,
            tc.tile_pool(name="lhs", bufs=3) as lhsp,
            tc.tile_pool(name="ps", bufs=2, space="PSUM") as psp,
            tc.tile_pool(name="sb", bufs=3) as sbp,
        ):
            w1_sb = constp.tile([IN, 80], F32)
            nc.sync.dma_start(out=w1_sb[:], in_=W1cat[:])
            G = 4
            ntile = NPC // P                       # 98
            for g in range((ntile + G - 1) // G):
                nblk = min(G, ntile - g * G)
                lhs = lhsp.tile([IN, P * G], F32, tag="lhs")
                nc.sync.dma_start(
                    out=lhs[:, :nblk * P],
                    in_=featownT[:, g * G * P: (g * G + nblk) * P])
                ps = psp.tile([P, G * 80], F32, tag="ps")
                for m in range(nblk):
                    nc.tensor.matmul(
                        out=ps[:, m * 80:(m + 1) * 80],
                        lhsT=lhs[:, m * P:(m + 1) * P],
                        rhs=w1_sb[:],
                        start=True, stop=True)
                sb = sbp.tile([P, G * 80], F32, tag="sb")
                nc.vector.tensor_copy(out=sb[:, :nblk * 80], in_=ps[:, :nblk * 80])
                dst = bass.AP(k1tab[:, :].tensor, g * G * P * 80,
                              [[80, P], [P * 80, nblk], [1, 80]])
                nc.sync.dma_start(out=dst, in_=sb[:, :nblk * 80])
    _split_multiwait(nc)
    return nc


# ---------------------------------------- K2: layer-1 edge phase + h2 rows
def _build_k2(db, S, rep=1):
    boff = np.concatenate([[0], np.cumsum(db)]).astype(np.int64)
    nc = bass.Bass()
    exp1 = nc.declare_dram_parameter("exp1", [P, S * R1], F32, isOutput=False)
    adstT = nc.declare_dram_parameter("adstT", [P, NB * 8], F32, isOutput=False)
    W2cat = nc.declare_dram_parameter("W2cat", [64, 66], F32, isOutput=False)
    b1b = nc.declare_dram_parameter("b1b", [P, 64], F32, isOutput=False)
    ident = nc.declare_dram_parameter("ident", [P, P], F32, isOutput=False)
    k1out = nc.declare_dram_parameter("k1out", [NPC, 66], F32, isOutput=True)

    with tile.TileContext(nc) as tc:
        with (
            tc.tile_pool(name="const", bufs=1) as constp,
            tc.tile_pool(name="gth", bufs=3) as gthp,
            tc.tile_pool(name="work", bufs=2) as workp,
            tc.tile_pool(name="ps", bufs=2, space="PSUM") as psp,
        ):
            w2_sb = constp.tile([64, 66], F32)
            nc.sync.dma_start(out=w2_sb[:], in_=W2cat[:])
            b1_sb = constp.tile([P, 64], F32)
            nc.sync.dma_start(out=b1_sb[:], in_=b1b[:])
            id_sb = constp.tile([P, P], F32)
            nc.sync.dma_start(out=id_sb[:], in_=ident[:])
            ad_sb = constp.tile([P, NB * 8], F32)
            nc.sync.dma_start(out=ad_sb[:], in_=adstT[:])

            for _rep in range(rep):
              for b in range(NB):
                d = int(db[b])
                off = int(boff[b])
                gth = gthp.tile([P, d * R1], F32, tag="gth")
                nc.sync.dma_start(
                    out=gth[:], in_=exp1[:, off * R1:(off + d) * R1])
                ga = gth[:, :]
                e = workp.tile([P, d * 8], F32, tag="e")
                nc.vector.tensor_tensor(
                    out=e[:],
                    in0=_ap(ga, [[R1, d], [1, 8]], offset=64),
                    in1=_ap(ad_sb[:, :], [[0, d], [1, 8]], offset=b * 8),
                    op=Alu.add)
                e2 = workp.tile([P, d * 8], F32, tag="e2")
                nc.vector.tensor_scalar_mul(e2[:], e[:], NEG)
                nc.vector.tensor_tensor(out=e2[:], in0=e[:], in1=e2[:], op=Alu.max)
                ex = workp.tile([P, d * 8], F32, tag="ex")
                nc.scalar.activation(out=ex[:], in_=e2[:], func=Act.Exp)
                den = workp.tile([P, 8], F32, tag="den")
                nc.vector.tensor_reduce(
                    out=den[:], in_=_ap(ex[:, :], [[1, 8], [8, d]]),
                    axis=Ax.X, op=Alu.add)
                rden = workp.tile([P, 8], F32, tag="rden")
                nc.vector.reciprocal(rden[:], den[:])
                msg = workp.tile([P, d * 64], F32, tag="msg")
                nc.vector.tensor_tensor(
                    out=msg[:],
                    in0=_ap(ga, [[R1, d], [1, 64]]),
                    in1=_ap(ex[:, :], [[8, d], [1, 8], [0, 8]]),
                    op=Alu.mult)
                oraw = workp.tile([P, 64], F32, tag="oraw")
                nc.vector.tensor_reduce(
                    out=oraw[:], in_=_ap(msg[:, :], [[1, 64], [64, d]]),
                    axis=Ax.X, op=Alu.add)
                x = workp.tile([P, 64], F32, tag="x")
                nc.vector.tensor_tensor(
                    out=x[:], in0=oraw[:],
                    in1=_ap(rden[:, :], [[1, 8], [0, 8]]), op=Alu.mult)
                nc.vector.tensor_tensor(out=x[:], in0=x[:], in1=b1_sb[:], op=Alu.add)
                # elu = exp(min(x,0)) - 1 + max(x,0)
                mn = workp.tile([P, 64], F32, tag="mn")
                nc.vector.tensor_scalar_min(mn[:], x[:], 0.0)
                em = workp.tile([P, 64], F32, tag="em")
                nc.scalar.activation(out=em[:], in_=mn[:], func=Act.Exp)
                nc.vector.tensor_scalar_max(x[:], x[:], 0.0)
                nc.vector.tensor_tensor(out=x[:], in0=x[:], in1=em[:], op=Alu.add)
                nc.vector.tensor_scalar_add(x[:], x[:], -1.0)
                # layer-2 factor row: [x @ W2 | x @ wa_src2 | x @ wa_dst2]
                xt_ps = psp.tile([64, P], F32, tag="xt")
                nc.tensor.transpose(out=xt_ps[:], in_=x[:], identity=id_sb[:])
                xt = workp.tile([64, P], F32, tag="xts")
                nc.scalar.activation(out=xt[:], in_=xt_ps[:], func=Act.Copy)
                h2 = psp.tile([P, 66], F32, tag="h2")
                nc.tensor.matmul(out=h2[:], lhsT=xt[:], rhs=w2_sb[:],
                                 start=True, stop=True)
                h2s = workp.tile([P, 66], F32, tag="h2s")
                nc.scalar.activation(out=h2s[:], in_=h2[:], func=Act.Copy)
                nc.sync.dma_start(out=k1out[b * P:(b + 1) * P, :], in_=h2s[:])
    _split_multiwait(nc)
    return nc


# ------------------------------------------------- K3: layer-2 edge phase
def _build_k3(db, S, rep=1):
    boff = np.concatenate([[0], np.cumsum(db)]).astype(np.int64)
    nc = bass.Bass()
    exp2 = nc.declare_dram_parameter("exp2", [P, S * R2], F32, isOutput=False)
    adst2T = nc.declare_dram_parameter("adst2T", [P, NB], F32, isOutput=False)
    b2b = nc.declare_dram_parameter("b2b", [P, C2], F32, isOutput=False)
    scout = nc.declare_dram_parameter("scout", [NPC, C2], F32, isOutput=True)

    with tile.TileContext(nc) as tc:
        with (
            tc.tile_pool(name="const", bufs=1) as constp,
            tc.tile_pool(name="gth", bufs=4) as gthp,
            tc.tile_pool(name="work", bufs=2) as workp,
        ):
            b2_sb = constp.tile([P, C2], F32)
            nc.sync.dma_start(out=b2_sb[:], in_=b2b[:])
            ad_sb = constp.tile([P, NB], F32)
            nc.sync.dma_start(out=ad_sb[:], in_=adst2T[:])

            for _rep in range(rep):
              for b in range(NB):
                d = int(db[b])
                off = int(boff[b])
                gth = gthp.tile([P, d * R2], F32, tag="gth")
                nc.sync.dma_start(
                    out=gth[:], in_=exp2[:, off * R2:(off + d) * R2])
                ga = gth[:, :]
                e = workp.tile([P, d], F32, tag="e")
                nc.vector.tensor_tensor(
                    out=e[:],
                    in0=_ap(ga, [[R2, d]], offset=64),
                    in1=_ap(ad_sb[:, :], [[0, d]], offset=b),
                    op=Alu.add)
                e2 = workp.tile([P, d], F32, tag="e2")
                nc.vector.tensor_scalar_mul(e2[:], e[:], NEG)
                nc.vector.tensor_tensor(out=e2[:], in0=e[:], in1=e2[:], op=Alu.max)
                ex = workp.tile([P, d], F32, tag="ex")
                nc.scalar.activation(out=ex[:], in_=e2[:], func=Act.Exp)
                den = workp.tile([P, 1], F32, tag="den")
                nc.vector.tensor_reduce(out=den[:], in_=ex[:], axis=Ax.X, op=Alu.add)
                rden = workp.tile([P, 1], F32, tag="rden")
                nc.vector.reciprocal(rden[:], den[:])
                msg = workp.tile([P, d * 64], F32, tag="msg")
                nc.vector.tensor_tensor(
                    out=msg[:],
                    in0=_ap(ga, [[R2, d], [1, 64]]),
                    in1=_ap(ex[:, :], [[1, d], [0, 64]]),
                    op=Alu.mult)
                oraw = workp.tile([P, 64], F32, tag="oraw")
                nc.vector.tensor_reduce(
                    out=oraw[:], in_=_ap(msg[:, :], [[1, 64], [64, d]]),
                    axis=Ax.X, op=Alu.add)
                sc = workp.tile([P, 64], F32, tag="sc")
                nc.scalar.activation(out=sc[:], in_=oraw[:], func=Act.Copy,
                                     scale=rden[:, 0:1])
                nc.vector.tensor_tensor(out=sc[:], in0=sc[:], in1=b2_sb[:], op=Alu.add)
                nc.sync.dma_start(out=scout[b * P:(b + 1) * P, :], in_=sc[:])
    _split_multiwait(nc)
    return nc


# ---------------------------------------------------------------- entry
def kernel(nodes, feat, edge_index, mask, label,
           W1, att_src1, att_dst1, b1, W2, att_src2, att_dst2, b2):
    prep = _host_prep(feat, edge_index, W1, att_src1, att_dst1,
                      W2, att_src2, att_dst2, b1, b2)
    db, S, slots = prep["db"], prep["S"], prep["slots"]
    featPT = np.ascontiguousarray(prep["featP"].T)
    ident = np.eye(P, dtype=np.float32)

    # K1: per-node [h | a_src | a_dst] rows (each core computes its own slice)
    nc1 = _build_k1()
    maps1 = [{
        "featownT": np.ascontiguousarray(featPT[:, c * NPC:(c + 1) * NPC]),
        "W1cat": prep["W1cat"],
    } for c in range(NCORES)]
    res1 = run_bass_kernel_spmd(nc1, maps1, list(range(NCORES)))
    k1tab = np.concatenate([res1.results[c]["k1tab"] for c in range(NCORES)], 0)

    # host staging: expand [h|a_src] rows to edge slots (dst-partitioned)
    table1 = np.ascontiguousarray(k1tab[:, :R1])
    table1[PAD0] = 0.0
    table1[PADN, :64] = 0.0
    table1[PADN, 64:] = NEG_BIG
    adstT = np.stack([
        np.ascontiguousarray(
            k1tab[c * NPC:(c + 1) * NPC, 72:80].reshape(NB, P, 8)
            .transpose(1, 0, 2).reshape(P, NB * 8))
        for c in range(NCORES)], 0)

    nc2 = _build_k2(db, S)
    maps2 = [{
        "exp1": table1[slots[c]].reshape(P, S * R1),
        "adstT": adstT[c],
        "W2cat": prep["W2cat"],
        "b1b": prep["b1b"],
        "ident": ident,
    } for c in range(NCORES)]
    res2 = run_bass_kernel_spmd(nc2, maps2, list(range(NCORES)))
    k1out = np.concatenate([res2.results[c]["k1out"] for c in range(NCORES)], 0)

    table2 = np.ascontiguousarray(k1out[:, :R2])
    table2[PAD0] = 0.0
    table2[PADN, :64] = 0.0
    table2[PADN, 64] = NEG_BIG
    adst2 = k1out[:, 65]

    nc3 = _build_k3(db, S)
    maps3 = [{
        "exp2": table2[slots[c]].reshape(P, S * R2),
        "adst2T": np.ascontiguousarray(
            adst2[c * NPC:(c + 1) * NPC].reshape(NB, P).T),
        "b2b": prep["b2b"],
    } for c in range(NCORES)]
    res3 = run_bass_kernel_spmd(nc3, maps3, list(range(NCORES)))
    scoresP = np.concatenate([res3.results[c]["scout"] for c in range(NCORES)], 0)
    scores = scoresP[prep["perm"]]

    # host head: log_softmax + masked-mean NLL + argmax
    m = scores.max(axis=1, keepdims=True)
    lse = (np.log(np.sum(np.exp(scores - m), axis=1, dtype=np.float32))
           + m[:, 0]).astype(np.float32)
    lab = np.asarray(label).astype(np.int32)
    nll = lse - scores[np.arange(N), lab]
    mf = np.asarray(mask).astype(np.float32)
    loss = np.float32(np.sum(nll * mf, dtype=np.float32)
                      / np.sum(mf, dtype=np.float32))
    pred = np.argmax(scores, axis=1).astype(np.int32)
    return loss, pred, lab
